# revision 12
# baseline (speedup 1.0000x reference)
"""v5: fp16 feature-major LSTM cell kernel, single-term matmuls, wide DMA ops.

Host prep per shard (part of the sharding strategy; QR = R/4 rows per
chunk class q, class q = contiguous row block [q*QR, (q+1)*QR)):
  xh2 [196, QR] f16 : row k*4+q = augmented input row k ([x.T; h.T; ones])
                      of class q
  c2  [128, QR] f16 : row 32q+h = c.T row h of class q (partition layout)
  w   [49, 128] f16 : [Wx; Wh; b] fused gate weights, columns [i|f|g|o]
Output out2 [256, QR] f16 : row (32q+h)*2+z, z=0 -> h_new.T, z=1 -> c_new.T.
These layouts keep every DMA AP 2D so descriptor size is controlled
explicitly with max_dma_last_dim (4KB descriptors).

Device layout: chunk class q ∈ 0..3 = contiguous row block [q*R/4, (q+1)*R/4).
Group slices live at partition p = 32q + h (layout-L, [128, 512] tiles).

DMA: HWDGE stripes each op's descriptors over DMA engines in 32-desc chunks
restarting at engine 0 every op, so an op wants >= 512 descriptors (ideally
a multiple of 512) to use all 16 engines evenly. Batch = 8 groups (16384
rows): xh loads are one 392x4KB-descriptor op per batch, c loads cover two
batches with one 512x4KB op, and each batch's h/c outputs interleave into
one dram tensor (out2) so the store is one op. Loads go on the sync HWDGE
queue; stores on gpsimd SWDGE (separate queue so store semaphore waits
can't block load prefetch, and the ACT engine stays clean; SWDGE spreads
its ~1KB packets over all 16 engines regardless of descriptor count).
"""

import sys

if "/opt/trn_rl_repo" not in sys.path:
    sys.path.insert(0, "/opt/trn_rl_repo")

import numpy as np

import bass_rust
import concourse.bass as bass
import concourse.tile as tile
from concourse import mybir

F32 = mybir.dt.float32
F16 = mybir.dt.float16
AF = mybir.ActivationFunctionType

B = 1048576
N_CORES = 8
R = B // N_CORES
IN_DIM, H_DIM = 16, 32
XH = IN_DIM + H_DIM
K_AUG = XH + 1  # 49
G4 = 4 * H_DIM  # 128
P = 128
TF = 512  # rows per chunk slice (matmul free dim)
NQ = 4  # chunk classes
NG_B = 8  # groups per batch
BW = NG_B * TF  # 4096 cols per batch tile per class
QR = R // NQ  # rows per chunk class, 32768

# gate column ranges in w: [i | f | g | o]
GATE_COLS = {"i": (0, 32), "f": (32, 64), "g": (64, 96), "o": (96, 128)}


def _split_waits(nc, max_waits=1):
    """Walrus codegen allows at most one semaphore wait per instruction.

    Move excess waits onto preceding same-engine EventSemaphore (pure wait)
    instructions; program order on the engine queue makes this equivalent.
    """
    n = 0
    for f in nc.m.functions:
        for blk in f.blocks:
            insts = blk.instructions
            new = []
            for inst in insts:
                si = inst.sync_info
                waits = list(si.on_wait) if si and si.on_wait else []
                if len(waits) > max_waits:
                    excess, keep = waits[:-max_waits], waits[-max_waits:]
                    for j in range(0, len(excess), max_waits):
                        nop = mybir.InstEventSemaphore(
                            name=f"{inst.name}-tw{j}", ins=[], outs=[]
                        )
                        nop.engine = inst.engine
                        nop.sync_info = bass_rust.SyncInfo(
                            on_wait=excess[j : j + max_waits], on_update=[]
                        )
                        new.append(nop)
                        n += 1
                    si.on_wait = keep
                    inst.sync_info = si
                new.append(inst)
            insts[:] = new
    return n


def build_nc(rows=R):
    assert rows % (NQ * 2 * BW) == 0
    nb = rows // (NQ * BW)  # batches, 8

    qr = rows // NQ

    nc = bass.Bass()
    xh2 = nc.dram_tensor("xh2", [K_AUG * NQ, qr], F16, kind="ExternalInput")
    c2 = nc.dram_tensor("c2", [P, qr], F16, kind="ExternalInput")
    w = nc.dram_tensor("w", [K_AUG, G4], F16, kind="ExternalInput")
    out2 = nc.dram_tensor("out2", [2 * P, qr], F16, kind="ExternalOutput")

    with tile.TileContext(nc) as tc:
        with (
            tc.tile_pool(name="const", bufs=1) as constp,
            tc.tile_pool(name="io", bufs=2) as iop,
            tc.tile_pool(name="acc", bufs=2) as accp,
            tc.tile_pool(name="work", bufs=3) as workp,
            tc.tile_pool(name="psum", bufs=2, space="PSUM") as psump,
        ):
            w_sb = constp.tile([K_AUG, G4], F16, tag="w")
            nc.sync.dma_start(w_sb[:], w[:])

            for ci in range(nb // 2):
                # c for two batches: 128*4 = 512 descriptors of 4KB
                c_sb = iop.tile([P, 2 * BW], F16, tag="c")
                nc.sync.dma_start(
                    c_sb[:],
                    c2[:, 2 * ci * BW : (2 * ci + 2) * BW],
                    max_dma_last_dim=2048,
                )

                for half in range(2):
                    bi = 2 * ci + half
                    # xh for one batch: 196*2 = 392 descriptors of 4KB
                    xh_sb = iop.tile([K_AUG, NQ, BW], F16, tag="xh")
                    nc.sync.dma_start(
                        xh_sb[:],
                        xh2[:, bi * BW : (bi + 1) * BW],
                        max_dma_last_dim=2048,
                    )

                    # acc free dims (z, col): z=0 -> h_new, z=1 -> c_new
                    acc = accp.tile([P, 2, BW], F16, tag="acc")

                    for gi in range(NG_B):
                        col = gi * TF
                        ifo_ps = psump.tile([P, 3, TF], F32, tag="ifo")
                        g_ps = psump.tile([P, TF], F32, tag="g")

                        def dest_ap(gate, q):
                            if gate == "i":
                                return ifo_ps[32 * q : 32 * q + 32, 0, :]
                            if gate == "f":
                                return ifo_ps[32 * q : 32 * q + 32, 1, :]
                            if gate == "o":
                                return ifo_ps[32 * q : 32 * q + 32, 2, :]
                            return g_ps[32 * q : 32 * q + 32, :]

                        for gate in ("i", "f", "g", "o"):
                            c0, c1 = GATE_COLS[gate]
                            for q in range(NQ):
                                nc.tensor.matmul(
                                    dest_ap(gate, q),
                                    w_sb[:, c0:c1],
                                    xh_sb[:, q, col : col + TF],
                                    start=True,
                                    stop=True,
                                    tile_position=(0, 32 * q),
                                )

                        ifo_sb = workp.tile([P, 3, TF], F16, tag="ifo_sb")
                        nc.scalar.activation(ifo_sb[:], ifo_ps[:], AF.Sigmoid)
                        g_sb = workp.tile([P, TF], F16, tag="g_sb")
                        nc.scalar.activation(g_sb[:], g_ps[:], AF.Tanh)

                        m1 = workp.tile([P, TF], F16, tag="m1")
                        nc.vector.tensor_mul(m1[:], ifo_sb[:, 0, :], g_sb[:])
                        m2 = workp.tile([P, TF], F16, tag="m2")
                        nc.vector.tensor_mul(
                            m2[:],
                            ifo_sb[:, 1, :],
                            c_sb[:, half * BW + col : half * BW + col + TF],
                        )
                        nc.vector.tensor_add(
                            acc[:, 1, col : col + TF], m1[:], m2[:]
                        )
                        tc_sb = workp.tile([P, TF], F16, tag="tc")
                        nc.scalar.activation(
                            tc_sb[:], acc[:, 1, col : col + TF], AF.Tanh
                        )
                        nc.vector.tensor_mul(
                            acc[:, 0, col : col + TF], ifo_sb[:, 2, :], tc_sb[:]
                        )

                    # one store op per batch: 256 runs (acc (p, z) dims
                    # merge against out2's row dim), SWDGE packets ~1KB
                    nc.gpsimd.dma_start(
                        out2[:, bi * BW : (bi + 1) * BW], acc[:]
                    )

    _split_waits(nc)
    return nc


def host_prep(x, h, c, Wx, Wh, b):
    """Build full-batch host arrays (sharding slices columns)."""
    n = x.shape[0]
    A = np.empty((K_AUG, n), dtype=np.float16)
    A[0:IN_DIM] = np.asarray(x).T
    A[IN_DIM:XH] = np.asarray(h).T
    A[XH] = 1.0
    W = np.concatenate(
        [np.asarray(Wx), np.asarray(Wh), np.asarray(b)[None, :]], axis=0
    ).astype(np.float16)  # [49, 128]
    cTfull = np.asarray(c).T.astype(np.float16)  # [32, n]
    return A, cTfull, W


def _shard_maps(A, cTfull, W, rows, n_cores):
    """Per-core input dicts in the device 2D layouts."""
    qr = rows // NQ
    maps = []
    for i in range(n_cores):
        sl = slice(i * rows, (i + 1) * rows)
        Ac = A[:, sl].reshape(K_AUG, NQ, qr)  # row k*4+q
        xh2 = np.ascontiguousarray(Ac.reshape(K_AUG * NQ, qr))
        cc = cTfull[:, sl].reshape(H_DIM, NQ, qr)
        c2 = np.ascontiguousarray(
            cc.transpose(1, 0, 2).reshape(P, qr)  # row 32q+h
        )
        maps.append({"xh2": xh2, "c2": c2, "w": W})
    return maps


_NC_CACHE = {}


def _get_nc(rows=R):
    if rows not in _NC_CACHE:
        _NC_CACHE[rows] = build_nc(rows)
    return _NC_CACHE[rows]


def run(x, h, c, Wx, Wh, b, trace=False, rows=R, n_cores=N_CORES):
    """Shard, execute on the 8 cores, gather. Returns (h_new, c_new, results)."""
    from concourse.bass_utils import run_bass_kernel_spmd

    A, cTfull, W = host_prep(x, h, c, Wx, Wh, b)
    nc = _get_nc(rows)
    in_maps = _shard_maps(A, cTfull, W, rows, n_cores)
    res = run_bass_kernel_spmd(nc, in_maps, list(range(n_cores)), trace=trace)
    n = rows * n_cores
    qr = rows // NQ
    h_new = np.empty((n, H_DIM), dtype=np.float32)
    c_new = np.empty((n, H_DIM), dtype=np.float32)
    for i, r in enumerate(res.results):
        # out2 row (32q+h)*2+z -> [q, h, z, col]
        o = r["out2"].reshape(NQ, H_DIM, 2, qr)
        for q in range(NQ):
            sl = slice(i * rows + q * qr, i * rows + (q + 1) * qr)
            h_new[sl] = o[q, :, 0, :].T.astype(np.float32)
            c_new[sl] = o[q, :, 1, :].T.astype(np.float32)
    return h_new, c_new, res


def kernel(x, h, c, Wx, Wh, b):
    h_new, c_new, _ = run(x, h, c, Wx, Wh, b)
    return h_new, c_new


# revision 20
# speedup vs baseline: 1.1253x; 1.1253x over previous
"""v6: fp16 LSTM cell kernel — g-fold sigmoid, SWDGE DMA, pipelined ACT tail.

Host prep per shard (QR = R/4 rows per chunk class q; class q = contiguous
row block [q*QR, (q+1)*QR)):
  xh2 [196, QR] f16 : row k*4+q = augmented input row k ([x.T; h.T; ones])
                      of class q
  c2  [128, QR] f16 : row 32q+h = c.T row h of class q (partition layout)
  w   [49, 128] f16 : [Wx; Wh; b] fused gate weights, columns [i|f|g|o],
                      g columns (64:96) PRE-DOUBLED on host (g-fold)
Output out2 [256, QR] f16 : row (32q+h)*2+z, z=0 -> h_new.T, z=1 -> c_new.T.

Device layout: group slices live at partition p = 32q + h (layout-L,
[128, 512] tiles). All four gate pre-activations for a group land in one
PSUM tile [128, 4, 512] (banks i, f, o, 2g), so ONE sigmoid op activates
everything: tanh(g) is reconstructed as 2*sigmoid(2g) - 1 on the DVE
(the x2 lives in the host weights). Per group:
  ACT: s = sigmoid(ps)            [128, 4, 512]
  DVE: tg = 2*s_g - 1; m1 = s_i * tg; m2 = s_f * c; cn = m1 + m2 -> acc
  per PAIR of groups (shifted 2 groups later to avoid ACT stalls):
  ACT: tc = tanh(cn pair)         [128, 1024]
  DVE: hn = s_o * tc -> acc       (per group)

DMA: everything on the gpsimd SWDGE queue — with 2D dram layouts its ~8KB
packets spread evenly over all 16 DMA engines at ~26 B/ns (HWDGE queues
only reach 7 engines). Loads for batch b+1 are issued BEFORE batch b's
compute so they sit ahead of batch b's store in the in-order queue.
"""

import sys

if "/opt/trn_rl_repo" not in sys.path:
    sys.path.insert(0, "/opt/trn_rl_repo")

import numpy as np

import bass_rust
import concourse.bass as bass
import concourse.tile as tile
from concourse import mybir

F32 = mybir.dt.float32
F16 = mybir.dt.float16
AF = mybir.ActivationFunctionType

B = 1048576
N_CORES = 8
R = B // N_CORES
IN_DIM, H_DIM = 16, 32
XH = IN_DIM + H_DIM
K_AUG = XH + 1  # 49
G4 = 4 * H_DIM  # 128
P = 128
TF = 512  # rows per chunk slice (matmul free dim)
NQ = 4  # chunk classes
NG_B = 8  # groups per batch
BW = NG_B * TF  # 4096 cols per batch tile per class
QR = R // NQ  # rows per chunk class, 32768

# gate -> (weight column range, psum bank)
GATES = {"i": (0, 32, 0), "f": (32, 64, 1), "g": (64, 96, 3), "o": (96, 128, 2)}


def _split_waits(nc, max_waits=1):
    """Walrus codegen allows at most one semaphore wait per instruction.

    Move excess waits onto preceding same-engine EventSemaphore (pure wait)
    instructions; program order on the engine queue makes this equivalent.
    """
    n = 0
    for f in nc.m.functions:
        for blk in f.blocks:
            insts = blk.instructions
            new = []
            for inst in insts:
                si = inst.sync_info
                waits = list(si.on_wait) if si and si.on_wait else []
                if len(waits) > max_waits:
                    excess, keep = waits[:-max_waits], waits[-max_waits:]
                    for j in range(0, len(excess), max_waits):
                        nop = mybir.InstEventSemaphore(
                            name=f"{inst.name}-tw{j}", ins=[], outs=[]
                        )
                        nop.engine = inst.engine
                        nop.sync_info = bass_rust.SyncInfo(
                            on_wait=excess[j : j + max_waits], on_update=[]
                        )
                        new.append(nop)
                        n += 1
                    si.on_wait = keep
                    inst.sync_info = si
                new.append(inst)
            insts[:] = new
    return n


def build_nc(rows=R):
    assert rows % (NQ * BW) == 0
    nb = rows // (NQ * BW)  # batches, 8
    qr = rows // NQ

    nc = bass.Bass()
    xh2 = nc.dram_tensor("xh2", [K_AUG * NQ, qr], F16, kind="ExternalInput")
    c2 = nc.dram_tensor("c2", [P, qr], F16, kind="ExternalInput")
    w = nc.dram_tensor("w", [K_AUG, G4], F16, kind="ExternalInput")
    out2 = nc.dram_tensor("out2", [2 * P, qr], F16, kind="ExternalOutput")

    with tile.TileContext(nc) as tc:
        with (
            tc.tile_pool(name="const", bufs=1) as constp,
            tc.tile_pool(name="io", bufs=2) as iop,
            tc.tile_pool(name="acc", bufs=3) as accp,
            # s tiles live until their pair's tail, 2 pairs + slop behind
            tc.tile_pool(name="sig", bufs=8) as sigp,
            tc.tile_pool(name="work", bufs=4) as workp,
            tc.tile_pool(name="pair", bufs=3) as pairp,
            tc.tile_pool(name="psum", bufs=2, space="PSUM") as psump,
        ):
            w_sb = constp.tile([K_AUG, G4], F16, tag="w")
            nc.gpsimd.dma_start(w_sb[:], w[:])

            xh_tiles = {}
            c_tiles = {}
            acc_tiles = {}
            s_tiles = {}
            pending = []  # (bi, g0) pair tails not yet emitted
            SHIFT_PAIRS = 2  # tails lag 2 pairs (4 groups) behind the adds

            def issue_loads(b, chunks=1):
                xh_sb = iop.tile([K_AUG, NQ, BW], F16, tag="xh")
                c_sb = iop.tile([P, BW], F16, tag="c")
                cw = BW // chunks
                for ch in range(chunks):
                    sl = slice(ch * cw, (ch + 1) * cw)
                    dsl = slice(b * BW + ch * cw, b * BW + (ch + 1) * cw)
                    nc.gpsimd.dma_start(xh_sb[:, :, sl], xh2[:, dsl])
                    nc.gpsimd.dma_start(c_sb[:, sl], c2[:, dsl])
                xh_tiles[b] = xh_sb
                c_tiles[b] = c_sb

            def tail_pair(b, g0):
                """tanh + hn for batch b groups g0, g0+1 (adds 4+ groups old).

                Triggers the batch's store once its last pair is done; the
                final batch stores per pair to shorten the pipeline drain.
                """
                last_batch = b == nb - 1
                acc = acc_tiles[b]
                col0 = g0 * TF
                tc_sb = pairp.tile([P, 2 * TF], F16, tag="tc")
                nc.scalar.activation(
                    tc_sb[:], acc[:, 1, col0 : col0 + 2 * TF], AF.Tanh
                )
                for gl in (g0, g0 + 1):
                    s_prev = s_tiles.pop((b, gl))
                    nc.vector.tensor_mul(
                        acc[:, 0, gl * TF : gl * TF + TF],
                        s_prev[:, 2, :],
                        tc_sb[:, (gl - g0) * TF : (gl - g0 + 1) * TF],
                    )
                if last_batch:
                    # drain the final batch per pair to shorten the tail
                    nc.gpsimd.dma_start(
                        out2[:, b * BW + col0 : b * BW + col0 + 2 * TF],
                        acc[:, :, col0 : col0 + 2 * TF],
                    )
                    if g0 == NG_B - 2:
                        acc_tiles.pop(b)
                elif g0 == NG_B - 2:
                    nc.gpsimd.dma_start(
                        out2[:, b * BW : (b + 1) * BW], acc[:]
                    )
                    acc_tiles.pop(b)

            # first batch loads split so group 0 can start early
            issue_loads(0, chunks=4)
            for bi in range(nb):
                if bi + 1 < nb:
                    issue_loads(bi + 1)
                xh_sb = xh_tiles.pop(bi)
                c_sb = c_tiles.pop(bi)

                # acc free dims (z, col): z=0 -> h_new, z=1 -> c_new
                acc = accp.tile([P, 2, BW], F16, tag="acc", name=f"acc{bi}")
                acc_tiles[bi] = acc

                for gi in range(NG_B):
                    col = gi * TF
                    ps = psump.tile([P, 4, TF], F32, tag="ps")

                    for gate, (c0, c1, bank) in GATES.items():
                        for q in range(NQ):
                            nc.tensor.matmul(
                                ps[32 * q : 32 * q + 32, bank, :],
                                w_sb[:, c0:c1],
                                xh_sb[:, q, col : col + TF],
                                start=True,
                                stop=True,
                                tile_position=(0, 32 * q),
                            )

                    s_sb = sigp.tile([P, 4, TF], F16, tag="s")
                    nc.scalar.activation(s_sb[:], ps[:], AF.Sigmoid)
                    s_tiles[(bi, gi)] = s_sb

                    tg = workp.tile([P, TF], F16, tag="tg")
                    nc.vector.tensor_scalar(
                        tg[:], s_sb[:, 3, :], 2.0, -1.0,
                        op0=mybir.AluOpType.mult, op1=mybir.AluOpType.add,
                    )
                    m1 = workp.tile([P, TF], F16, tag="m1")
                    nc.vector.tensor_mul(m1[:], s_sb[:, 0, :], tg[:])
                    m2 = workp.tile([P, TF], F16, tag="m2")
                    nc.vector.tensor_mul(
                        m2[:], s_sb[:, 1, :], c_sb[:, col : col + TF]
                    )
                    nc.vector.tensor_add(acc[:, 1, col : col + TF], m1[:], m2[:])

                    if gi % 2 == 1:
                        pending.append((bi, gi - 1))
                        if len(pending) > SHIFT_PAIRS:
                            tail_pair(*pending.pop(0))

            for b, g0 in pending:
                tail_pair(b, g0)

    _split_waits(nc)
    return nc


def host_prep(x, h, c, Wx, Wh, b):
    """Build full-batch host arrays (sharding slices columns)."""
    n = x.shape[0]
    A = np.empty((K_AUG, n), dtype=np.float16)
    A[0:IN_DIM] = np.asarray(x).T
    A[IN_DIM:XH] = np.asarray(h).T
    A[XH] = 1.0
    W = np.concatenate(
        [np.asarray(Wx), np.asarray(Wh), np.asarray(b)[None, :]], axis=0
    ).astype(np.float32)  # [49, 128]
    W[:, 64:96] *= 2.0  # g-fold: tanh(g) = 2*sigmoid(2g) - 1
    W = W.astype(np.float16)
    cTfull = np.asarray(c).T.astype(np.float16)  # [32, n]
    return A, cTfull, W


def _shard_maps(A, cTfull, W, rows, n_cores):
    """Per-core input dicts in the device 2D layouts."""
    qr = rows // NQ
    maps = []
    for i in range(n_cores):
        sl = slice(i * rows, (i + 1) * rows)
        Ac = A[:, sl].reshape(K_AUG, NQ, qr)  # row k*4+q
        xh2 = np.ascontiguousarray(Ac.reshape(K_AUG * NQ, qr))
        cc = cTfull[:, sl].reshape(H_DIM, NQ, qr)
        c2 = np.ascontiguousarray(
            cc.transpose(1, 0, 2).reshape(P, qr)  # row 32q+h
        )
        maps.append({"xh2": xh2, "c2": c2, "w": W})
    return maps


_NC_CACHE = {}


def _get_nc(rows=R):
    if rows not in _NC_CACHE:
        _NC_CACHE[rows] = build_nc(rows)
    return _NC_CACHE[rows]


def run(x, h, c, Wx, Wh, b, trace=False, rows=R, n_cores=N_CORES):
    """Shard, execute on the 8 cores, gather. Returns (h_new, c_new, results)."""
    from concourse.bass_utils import run_bass_kernel_spmd

    A, cTfull, W = host_prep(x, h, c, Wx, Wh, b)
    nc = _get_nc(rows)
    in_maps = _shard_maps(A, cTfull, W, rows, n_cores)
    res = run_bass_kernel_spmd(nc, in_maps, list(range(n_cores)), trace=trace)
    n = rows * n_cores
    qr = rows // NQ
    h_new = np.empty((n, H_DIM), dtype=np.float32)
    c_new = np.empty((n, H_DIM), dtype=np.float32)
    for i, r in enumerate(res.results):
        # out2 row (32q+h)*2+z -> [q, h, z, col]
        o = r["out2"].reshape(NQ, H_DIM, 2, qr)
        for q in range(NQ):
            sl = slice(i * rows + q * qr, i * rows + (q + 1) * qr)
            h_new[sl] = o[q, :, 0, :].T.astype(np.float32)
            c_new[sl] = o[q, :, 1, :].T.astype(np.float32)
    return h_new, c_new, res


def kernel(x, h, c, Wx, Wh, b):
    h_new, c_new, _ = run(x, h, c, Wx, Wh, b)
    return h_new, c_new


# revision 25
# speedup vs baseline: 1.3586x; 1.2073x over previous
"""v6: fp16 LSTM cell kernel — g-fold sigmoid, SWDGE DMA, pipelined ACT tail.

Host prep per shard (QR = R/4 rows per chunk class q; class q = contiguous
row block [q*QR, (q+1)*QR)):
  xh2 [196, QR] f16 : row k*4+q = augmented input row k ([x.T; h.T; ones])
                      of class q
  c2  [128, QR] f16 : row 32q+h = c.T row h of class q (partition layout)
  w   [49, 128] f16 : [Wx; Wh; b] fused gate weights, columns [i|f|g|o],
                      g columns (64:96) PRE-DOUBLED on host (g-fold)
Output out2 [256, QR] f16 : row (32q+h)*2+z, z=0 -> h_new.T, z=1 -> c_new.T.

Device layout: group slices live at partition p = 32q + h (layout-L,
[128, 512] tiles). All four gate pre-activations for a group land in one
PSUM tile [128, 4, 512] (banks i, f, o, 2g), so ONE sigmoid op activates
everything: tanh(g) is reconstructed as 2*sigmoid(2g) - 1 on the DVE
(the x2 lives in the host weights). Per group:
  ACT: s = sigmoid(ps)            [128, 4, 512]
  DVE: tg = 2*s_g - 1; m1 = s_i * tg; m2 = s_f * c; cn = m1 + m2 -> acc
  per PAIR of groups (shifted 2 groups later to avoid ACT stalls):
  ACT: tc = tanh(cn pair)         [128, 1024]
  DVE: hn = s_o * tc -> acc       (per group)

DMA: everything on the gpsimd SWDGE queue — with 2D dram layouts its ~8KB
packets spread evenly over all 16 DMA engines at ~26 B/ns (HWDGE queues
only reach 7 engines). Loads for batch b+1 are issued BEFORE batch b's
compute so they sit ahead of batch b's store in the in-order queue.
"""

import sys

if "/opt/trn_rl_repo" not in sys.path:
    sys.path.insert(0, "/opt/trn_rl_repo")

import numpy as np

import bass_rust
import concourse.bass as bass
import concourse.tile as tile
from concourse import mybir

F32 = mybir.dt.float32
F16 = mybir.dt.float16
AF = mybir.ActivationFunctionType

B = 1048576
N_CORES = 8
R = B // N_CORES
IN_DIM, H_DIM = 16, 32
XH = IN_DIM + H_DIM
K_AUG = XH + 1  # 49
G4 = 4 * H_DIM  # 128
P = 128
TF = 512  # rows per chunk slice (matmul free dim)
NQ = 4  # chunk classes
NG_B = 8  # groups per batch
BW = NG_B * TF  # 4096 cols per batch tile per class
QR = R // NQ  # rows per chunk class, 32768

# gate -> (weight column range, psum bank)
GATES = {"i": (0, 32, 0), "f": (32, 64, 1), "g": (64, 96, 3), "o": (96, 128, 2)}


def _split_waits(nc, max_waits=1):
    """Walrus codegen allows at most one semaphore wait per instruction.

    Move excess waits onto preceding same-engine EventSemaphore (pure wait)
    instructions; program order on the engine queue makes this equivalent.
    """
    n = 0
    for f in nc.m.functions:
        for blk in f.blocks:
            insts = blk.instructions
            new = []
            for inst in insts:
                si = inst.sync_info
                waits = list(si.on_wait) if si and si.on_wait else []
                if len(waits) > max_waits:
                    excess, keep = waits[:-max_waits], waits[-max_waits:]
                    for j in range(0, len(excess), max_waits):
                        nop = mybir.InstEventSemaphore(
                            name=f"{inst.name}-tw{j}", ins=[], outs=[]
                        )
                        nop.engine = inst.engine
                        nop.sync_info = bass_rust.SyncInfo(
                            on_wait=excess[j : j + max_waits], on_update=[]
                        )
                        new.append(nop)
                        n += 1
                    si.on_wait = keep
                    inst.sync_info = si
                new.append(inst)
            insts[:] = new
    return n


class _pe_model_fix:
    """Scale the scheduler cost model's PE timings to measured hardware.

    The stock TRN2 model charges fp16 matmuls 1.0 cycle/row @2.4GHz
    (213 ns per [49x32 @ 49x512] matmul); hardware measures ~119 ns.
    The tile list-scheduler orders engine queues around its simulated
    bottleneck, so with the stock model it thinks the PE is critical
    and serializes ACT behind the DVE chain, idling ACT ~1.5 us per
    group pair. Scaling PE_CYCLE for the duration of graph scheduling
    makes the simulation match hardware; restored afterwards.
    """

    FIELDS = ("PE_CYCLE", "PE_CYCLE_PSTATE_MID", "PE_CYCLE_PSTATE_LOW")

    def __init__(self, scale=0.56):
        self.scale = scale

    def __enter__(self):
        from concourse import hw_specs

        self.spec = hw_specs.TRN2Spec
        self.saved = {f: getattr(self.spec, f) for f in self.FIELDS}
        for f in self.FIELDS:
            setattr(self.spec, f, self.saved[f] * self.scale)

    def __exit__(self, *exc):
        for f, v in self.saved.items():
            setattr(self.spec, f, v)


def build_nc(rows=R):
    assert rows % (NQ * BW) == 0
    nb = rows // (NQ * BW)  # batches, 8
    qr = rows // NQ

    nc = bass.Bass()
    xh2 = nc.dram_tensor("xh2", [K_AUG * NQ, qr], F16, kind="ExternalInput")
    c2 = nc.dram_tensor("c2", [P, qr], F16, kind="ExternalInput")
    w = nc.dram_tensor("w", [K_AUG, G4], F16, kind="ExternalInput")
    out2 = nc.dram_tensor("out2", [2 * P, qr], F16, kind="ExternalOutput")

    with _pe_model_fix(), tile.TileContext(nc) as tc:
        with (
            tc.tile_pool(name="const", bufs=1) as constp,
            tc.tile_pool(name="io", bufs=2) as iop,
            tc.tile_pool(name="acc", bufs=3) as accp,
            # s tiles live until their pair's tail, 2 pairs + slop behind
            tc.tile_pool(name="sig", bufs=8) as sigp,
            tc.tile_pool(name="work", bufs=4) as workp,
            tc.tile_pool(name="pair", bufs=3) as pairp,
            tc.tile_pool(name="psum", bufs=2, space="PSUM") as psump,
        ):
            w_sb = constp.tile([K_AUG, G4], F16, tag="w")
            nc.sync.dma_start(w_sb[:], w[:])

            xh_tiles = {}
            c_tiles = {}
            acc_tiles = {}
            s_tiles = {}
            pending = []  # (bi, g0) pair tails not yet emitted
            SHIFT_PAIRS = 2  # tails lag 2 pairs (4 groups) behind the adds

            def issue_loads(b, chunks=1, eng=None):
                # first batch goes on the sync HWDGE queue: its DGE setup
                # is quicker and parallel to gpsimd's descriptor chain,
                # so group 0 data lands in SBUF sooner
                eng = eng or nc.gpsimd
                xh_sb = iop.tile([K_AUG, NQ, BW], F16, tag="xh")
                c_sb = iop.tile([P, BW], F16, tag="c")
                cw = BW // chunks
                for ch in range(chunks):
                    sl = slice(ch * cw, (ch + 1) * cw)
                    dsl = slice(b * BW + ch * cw, b * BW + (ch + 1) * cw)
                    eng.dma_start(xh_sb[:, :, sl], xh2[:, dsl])
                    eng.dma_start(c_sb[:, sl], c2[:, dsl])
                xh_tiles[b] = xh_sb
                c_tiles[b] = c_sb

            def tail_pair(b, g0):
                """tanh + hn for batch b groups g0, g0+1 (adds 4+ groups old).

                Triggers the batch's store once its last pair is done; the
                final batch stores per pair to shorten the pipeline drain.
                """
                last_batch = b == nb - 1
                acc = acc_tiles[b]
                col0 = g0 * TF
                tc_sb = pairp.tile([P, 2 * TF], F16, tag="tc")
                nc.scalar.activation(
                    tc_sb[:], acc[:, 1, col0 : col0 + 2 * TF], AF.Tanh
                )
                for gl in (g0, g0 + 1):
                    s_prev = s_tiles.pop((b, gl))
                    nc.vector.tensor_mul(
                        acc[:, 0, gl * TF : gl * TF + TF],
                        s_prev[:, 2, :],
                        tc_sb[:, (gl - g0) * TF : (gl - g0 + 1) * TF],
                    )
                if last_batch:
                    # drain the final batch per pair to shorten the tail
                    nc.gpsimd.dma_start(
                        out2[:, b * BW + col0 : b * BW + col0 + 2 * TF],
                        acc[:, :, col0 : col0 + 2 * TF],
                    )
                    if g0 == NG_B - 2:
                        acc_tiles.pop(b)
                elif g0 == NG_B - 2:
                    nc.gpsimd.dma_start(
                        out2[:, b * BW : (b + 1) * BW], acc[:]
                    )
                    acc_tiles.pop(b)

            # first batch loads split so group 0 can start early
            issue_loads(0, chunks=4, eng=nc.sync)
            for bi in range(nb):
                if bi + 1 < nb:
                    issue_loads(bi + 1)
                xh_sb = xh_tiles.pop(bi)
                c_sb = c_tiles.pop(bi)

                # acc free dims (z, col): z=0 -> h_new, z=1 -> c_new
                acc = accp.tile([P, 2, BW], F16, tag="acc", name=f"acc{bi}")
                acc_tiles[bi] = acc

                for gi in range(NG_B):
                    col = gi * TF
                    ps = psump.tile([P, 4, TF], F32, tag="ps")

                    for gate, (c0, c1, bank) in GATES.items():
                        for q in range(NQ):
                            nc.tensor.matmul(
                                ps[32 * q : 32 * q + 32, bank, :],
                                w_sb[:, c0:c1],
                                xh_sb[:, q, col : col + TF],
                                start=True,
                                stop=True,
                                tile_position=(0, 32 * q),
                            )

                    s_sb = sigp.tile([P, 4, TF], F16, tag="s")
                    nc.scalar.activation(s_sb[:], ps[:], AF.Sigmoid)
                    s_tiles[(bi, gi)] = s_sb

                    tg = workp.tile([P, TF], F16, tag="tg")
                    nc.vector.tensor_scalar(
                        tg[:], s_sb[:, 3, :], 2.0, -1.0,
                        op0=mybir.AluOpType.mult, op1=mybir.AluOpType.add,
                    )
                    m1 = workp.tile([P, TF], F16, tag="m1")
                    nc.vector.tensor_mul(m1[:], s_sb[:, 0, :], tg[:])
                    m2 = workp.tile([P, TF], F16, tag="m2")
                    nc.vector.tensor_mul(
                        m2[:], s_sb[:, 1, :], c_sb[:, col : col + TF]
                    )
                    nc.vector.tensor_add(acc[:, 1, col : col + TF], m1[:], m2[:])

                    if gi % 2 == 1:
                        pending.append((bi, gi - 1))
                        if len(pending) > SHIFT_PAIRS:
                            tail_pair(*pending.pop(0))

            for b, g0 in pending:
                tail_pair(b, g0)

    _split_waits(nc)
    return nc


def host_prep(x, h, c, Wx, Wh, b):
    """Build full-batch host arrays (sharding slices columns)."""
    n = x.shape[0]
    A = np.empty((K_AUG, n), dtype=np.float16)
    A[0:IN_DIM] = np.asarray(x).T
    A[IN_DIM:XH] = np.asarray(h).T
    A[XH] = 1.0
    W = np.concatenate(
        [np.asarray(Wx), np.asarray(Wh), np.asarray(b)[None, :]], axis=0
    ).astype(np.float32)  # [49, 128]
    W[:, 64:96] *= 2.0  # g-fold: tanh(g) = 2*sigmoid(2g) - 1
    W = W.astype(np.float16)
    cTfull = np.asarray(c).T.astype(np.float16)  # [32, n]
    return A, cTfull, W


def _shard_maps(A, cTfull, W, rows, n_cores):
    """Per-core input dicts in the device 2D layouts."""
    qr = rows // NQ
    maps = []
    for i in range(n_cores):
        sl = slice(i * rows, (i + 1) * rows)
        Ac = A[:, sl].reshape(K_AUG, NQ, qr)  # row k*4+q
        xh2 = np.ascontiguousarray(Ac.reshape(K_AUG * NQ, qr))
        cc = cTfull[:, sl].reshape(H_DIM, NQ, qr)
        c2 = np.ascontiguousarray(
            cc.transpose(1, 0, 2).reshape(P, qr)  # row 32q+h
        )
        maps.append({"xh2": xh2, "c2": c2, "w": W})
    return maps


_NC_CACHE = {}


def _get_nc(rows=R):
    if rows not in _NC_CACHE:
        _NC_CACHE[rows] = build_nc(rows)
    return _NC_CACHE[rows]


def run(x, h, c, Wx, Wh, b, trace=False, rows=R, n_cores=N_CORES):
    """Shard, execute on the 8 cores, gather. Returns (h_new, c_new, results)."""
    from concourse.bass_utils import run_bass_kernel_spmd

    A, cTfull, W = host_prep(x, h, c, Wx, Wh, b)
    nc = _get_nc(rows)
    in_maps = _shard_maps(A, cTfull, W, rows, n_cores)
    res = run_bass_kernel_spmd(nc, in_maps, list(range(n_cores)), trace=trace)
    n = rows * n_cores
    qr = rows // NQ
    h_new = np.empty((n, H_DIM), dtype=np.float32)
    c_new = np.empty((n, H_DIM), dtype=np.float32)
    for i, r in enumerate(res.results):
        # out2 row (32q+h)*2+z -> [q, h, z, col]
        o = r["out2"].reshape(NQ, H_DIM, 2, qr)
        for q in range(NQ):
            sl = slice(i * rows + q * qr, i * rows + (q + 1) * qr)
            h_new[sl] = o[q, :, 0, :].T.astype(np.float32)
            c_new[sl] = o[q, :, 1, :].T.astype(np.float32)
    return h_new, c_new, res


def kernel(x, h, c, Wx, Wh, b):
    h_new, c_new, _ = run(x, h, c, Wx, Wh, b)
    return h_new, c_new
